# revision 18
# baseline (speedup 1.0000x reference)
"""Trainium2 Bass kernel for nn_DSAM: background-prototype decomposition + LN +
semantic scores + 3 losses.  Data-parallel over batch B=16 across 8 NeuronCores
(2 batches/core).  Self-contained: hardcodes shapes; only imports concourse from
/opt/trn_rl_repo.
"""
import sys

if "/opt/trn_rl_repo" not in sys.path:
    sys.path.insert(0, "/opt/trn_rl_repo")

import numpy as np
import concourse.bass as bass
import concourse.mybir as mybir
import concourse.tile as tile
from concourse import bacc
from concourse.bass_utils import run_bass_kernel_spmd

FP32 = mybir.dt.float32
FP32R = mybir.dt.float32r
AF = mybir.ActivationFunctionType
ALU = mybir.AluOpType
AX = mybir.AxisListType

B, N, D, K = 16, 2048, 512, 8
NCORES = 8
PB = B // NCORES          # batches per core = 2
NC = N // 128             # 16 n-chunks of 128
DC = D // 128             # 4 d-chunks of 128
N4 = N // 512             # 4 n-chunks of 512
TAU = 0.07
EPS = 1e-6
LN_EPS = 1e-5
NORM_EPS = 1e-12

_PROG_CACHE = {}


def _build(alpha: float, general_ln: bool, phase: int = 99):
    """Build the per-core SPMD program (2 batches per core)."""
    nc = bacc.Bacc(None)

    f_raw = nc.dram_tensor("f_raw", [PB, N, D], FP32, kind="ExternalInput")
    qmat = nc.dram_tensor("qmat", [PB, D, 10], FP32, kind="ExternalInput")
    w2rep = nc.dram_tensor("w2rep", [PB, K, D], FP32, kind="ExternalInput")
    lnsc = nc.dram_tensor("lnsc", [PB, 2, 128], FP32, kind="ExternalInput")
    ident_d = nc.dram_tensor("ident", [128, 128], FP32, kind="ExternalInput")
    ones_d = nc.dram_tensor("ones128", [128, 1], FP32, kind="ExternalInput")
    orow8_d = nc.dram_tensor("onesrow8", [1, 8], FP32, kind="ExternalInput")
    if general_ln:
        lnw_d = nc.dram_tensor("lnwrep", [128, D], FP32, kind="ExternalInput")
        lnb_d = nc.dram_tensor("lnbrep", [128, D], FP32, kind="ExternalInput")
        fqn_d = nc.dram_tensor("fqnrep", [PB, 128, D], FP32, kind="ExternalInput")

    o_ffg = nc.dram_tensor("f_fg", [PB, N, D], FP32, kind="ExternalOutput")
    o_fbg = nc.dram_tensor("f_bg", [PB, N, D], FP32, kind="ExternalOutput")
    o_pbg = nc.dram_tensor("p_bg", [PB, K, D], FP32, kind="ExternalOutput")
    o_ssem = nc.dram_tensor("ssem", [PB, 128, NC], FP32, kind="ExternalOutput")
    o_stats = nc.dram_tensor("stats", [PB, 128, 32], FP32, kind="ExternalOutput")
    o_mats = nc.dram_tensor("mats", [PB, K, 16], FP32, kind="ExternalOutput")

    with tile.TileContext(nc) as tc:
        with (
            tc.tile_pool(name="big", bufs=2) as big,         # f_raw tiles (per batch)
            tc.tile_pool(name="wide", bufs=1) as wide,       # frT / numT / hsb / S1sb
            tc.tile_pool(name="small", bufs=2) as small,     # stats etc
            tc.tile_pool(name="stage", bufs=3) as stage,     # fbg/x/ffg staging
            tc.tile_pool(name="scr", bufs=2) as scr,         # square scratch
            tc.tile_pool(name="const", bufs=1) as constp,
            tc.tile_pool(name="psT", bufs=2, space="PSUM") as psT,     # transposes
            tc.tile_pool(name="psG", bufs=2, space="PSUM") as psG,     # groups
            tc.tile_pool(name="psS", bufs=2, space="PSUM") as psS,     # small psum
            tc.tile_pool(name="psF", bufs=2, space="PSUM") as psF,     # f_bg
        ):
            # ---- constants ----
            ident = constp.tile([128, 128], FP32, tag="ident")
            nc.sync.dma_start(ident[:], ident_d[:])
            ones128 = constp.tile([128, 1], FP32, tag="ones128")
            nc.sync.dma_start(ones128[:], ones_d[:])
            orow8 = constp.tile([1, 8], FP32, tag="orow8")
            nc.sync.dma_start(orow8[:], orow8_d[:])
            if general_ln:
                lnw = constp.tile([128, D], FP32, tag="lnw")
                nc.sync.dma_start(lnw[:], lnw_d[:])
                lnb = constp.tile([128, D], FP32, tag="lnb")
                nc.sync.dma_start(lnb[:], lnb_d[:])

            for b in range(PB):
                # ================= load =================
                fr = big.tile([128, NC, D], FP32, tag="fr")
                for c in range(NC):
                    nc.sync.dma_start(fr[:, c, :], f_raw[b, 128 * c : 128 * (c + 1), :])
                qm = small.tile([128, DC, 10], FP32R, tag="qm")
                for d in range(DC):
                    nc.gpsimd.dma_start(qm[:, d, :], qmat[b, 128 * d : 128 * (d + 1), :])
                w2r = small.tile([K, D], FP32, tag="w2r")
                nc.sync.dma_start(w2r[:], w2rep[b])
                lns = small.tile([128, 2], FP32, tag="lns")
                nc.sync.dma_start(lns[:], lnsc[b].rearrange("a p -> p a"))
                if general_ln:
                    fqn = big.tile([128, D], FP32, tag="fqn")
                    nc.sync.dma_start(fqn[:], fqn_d[b])

                # ====== sumsq_raw via ACT Square + accum (packed [128,NC]) ======
                ssq = small.tile([128, NC], FP32, tag="ssq")
                for c in range(NC):
                    sq = scr.tile([128, D], FP32, tag="sq")
                    nc.scalar.activation(sq[:], fr[:, c, :], AF.Square,
                                         accum_out=ssq[:, c : c + 1])

                # ================= f_rawT (PE transpose) =================
                frT = [wide.tile([128, N], FP32R, tag=f"frT{d}", name=f"frT{d}")
                       for d in range(DC)]
                for d in range(DC):
                    for c4 in range(N4):
                        pt = psT.tile([128, 512], FP32, tag="pt")
                        for j in range(4):
                            c = 4 * c4 + j
                            nc.tensor.transpose(
                                pt[:, 128 * j : 128 * (j + 1)],
                                fr[:, c, 128 * d : 128 * (d + 1)],
                                ident[:],
                            )
                        if c4 % 2 == 0:
                            nc.scalar.copy(frT[d][:, 512 * c4 : 512 * (c4 + 1)], pt[:])
                        else:
                            nc.vector.tensor_copy(frT[d][:, 512 * c4 : 512 * (c4 + 1)], pt[:])

                if phase <= 1:
                    continue
                # ================= S1 group: [10, N] = qmat.T @ f_raw.T ========
                S1sb = wide.tile([16, N], FP32, tag="S1sb")
                for c4 in range(N4):
                    g = psG.tile([16, 512], FP32, tag="grp")
                    for d in range(DC):
                        nc.tensor.matmul(
                            g[0:10, :], qm[:, d, :],
                            frT[d][:, 512 * c4 : 512 * (c4 + 1)],
                            start=(d == 0), stop=(d == DC - 1),
                        )
                    nc.vector.tensor_copy(S1sb[0:10, 512 * c4 : 512 * (c4 + 1)], g[0:10, :])

                # pack S1 -> [128, NC, 10]; exp rows 0:8 -> expS1t (fp32)
                pp = psS.tile([128, NC, 10], FP32, tag="ps", name="ppack")
                for c in range(NC):
                    nc.tensor.transpose(pp[:, c, :], S1sb[0:10, 128 * c : 128 * (c + 1)],
                                        ident[0:10, 0:10])
                expS1t = small.tile([128, NC, K], FP32R, tag="expS1t")
                nc.scalar.activation(expS1t[:], pp[:, :, 0:8], AF.Exp)
                expS1f = small.tile([128, NC, K], FP32, tag="expS1f")
                nc.scalar.activation(expS1f[:], pp[:, :, 0:8], AF.Exp)
                qrpack = small.tile([128, NC, 2], FP32, tag="qrpack")
                nc.vector.tensor_copy(qrpack[:], pp[:, :, 8:10])

                if phase <= 2:
                    continue
                # ================= p_bg (fp32 matmuls) =================
                pPB = psS.tile([K, D], FP32, tag="ps", name="pPB")
                pRS = psS.tile([K, 1], FP32, tag="ps", name="pRS")
                for c in range(NC):
                    fc = stage.tile([128, D], FP32R, tag="frN", name="frN", bufs=4)
                    if c % 2 == 0:
                        nc.vector.tensor_copy(fc[:], fr[:, c, :])
                    else:
                        nc.scalar.copy(fc[:], fr[:, c, :])
                    nc.tensor.matmul(pPB[:], expS1t[:, c, :], fc[:],
                                     start=(c == 0), stop=(c == NC - 1))
                for c in range(NC):
                    nc.tensor.matmul(pRS[:], expS1f[:, c, :], ones128[:],
                                     start=(c == 0), stop=(c == NC - 1))
                rinvK = small.tile([K, 1], FP32, tag="rinvK")
                nc.vector.reciprocal(rinvK[:], pRS[:])
                pbg = small.tile([K, D], FP32, tag="pbg")
                nc.vector.tensor_scalar_mul(pbg[:], pPB[:], rinvK[:])
                nc.sync.dma_start(o_pbg[b], pbg[:])
                if phase <= 21:
                    continue

                # ---- denom / rden / rnp / scaled copies ----
                scr8 = small.tile([K, D], FP32, tag="scr8")
                pn2 = small.tile([K, 1], FP32, tag="pn2")
                nc.vector.tensor_tensor(out=scr8[:], in0=pbg[:], in1=pbg[:], op=ALU.mult)
                nc.vector.reduce_sum(pn2[:], scr8[:], axis=AX.X)
                denomK = small.tile([K, 1], FP32, tag="denomK")
                nc.vector.tensor_scalar_add(denomK[:], pn2[:], EPS)
                rdenK = small.tile([K, 1], FP32, tag="rdenK")
                nc.vector.reciprocal(rdenK[:], denomK[:])
                # rnp = 1/max(sqrt(pn2), NORM_EPS)
                t8a = small.tile([K, 1], FP32, tag="t8a")
                nc.scalar.activation(t8a[:], pn2[:], AF.Sqrt)
                t8b = small.tile([K, 1], FP32, tag="t8b")
                nc.vector.tensor_scalar_max(t8b[:], t8a[:], NORM_EPS)
                rnpK = small.tile([K, 1], FP32, tag="rnpK")
                nc.vector.reciprocal(rnpK[:], t8b[:])
                if phase <= 22:
                    continue

                pbg_s = small.tile([K, D], FP32R, tag="pbg_s")
                nc.vector.tensor_scalar_mul(pbg_s[:], pbg[:], rdenK[:])
                npbg = small.tile([K, D], FP32R, tag="npbg")
                nc.vector.tensor_scalar_mul(npbg[:], pbg[:], rnpK[:])

                # transpose p_bg and np_bg -> nlhsT [128, DC, 16]
                pT = psS.tile([128, DC, 16], FP32, tag="ps", name="pT")
                for d in range(DC):
                    nc.tensor.transpose(pT[:, d, 0:8], pbg[:, 128 * d : 128 * (d + 1)],
                                        ident[0:8, 0:8])
                    nc.tensor.transpose(pT[:, d, 8:16],
                                        npbg[:, 128 * d : 128 * (d + 1)].bitcast(FP32),
                                        ident[0:8, 0:8])
                nlhsT = small.tile([128, DC, 16], FP32R, tag="nlhsT")
                nc.vector.tensor_copy(nlhsT[:], pT[:])

                if phase <= 3:
                    continue
                # ================= numer group: [16, N] =================
                numT = wide.tile([16, N], FP32R, tag="numT")
                for c4 in range(N4):
                    g = psG.tile([16, 512], FP32, tag="grp")
                    for d in range(DC):
                        nc.tensor.matmul(
                            g[:], nlhsT[:, d, :],
                            frT[d][:, 512 * c4 : 512 * (c4 + 1)],
                            start=(d == 0), stop=(d == DC - 1),
                        )
                    nc.scalar.copy(numT[:, 512 * c4 : 512 * (c4 + 1)], g[:])

                # ================= hT group lhsT assembly [8, 18] ============
                pG = psS.tile([K, K], FP32, tag="ps", name="pG")
                for d in range(DC):
                    nc.tensor.matmul(pG[:], nlhsT[:, d, 0:8], nlhsT[:, d, 0:8],
                                     start=(d == 0), stop=(d == DC - 1))
                G1 = small.tile([K, K], FP32, tag="G1")
                nc.vector.tensor_scalar_mul(G1[:], pG[:], rdenK[:])  # row scale
                # rdenT replicated [8,8] via transpose + ones-bcast matmul
                pRT = psS.tile([1, 8], FP32, tag="ps", name="pRT")
                nc.tensor.transpose(pRT[:], rdenK[:], ident[0:8, 0:8])
                rdT = small.tile([1, 8], FP32, tag="rdT")
                nc.vector.tensor_copy(rdT[:], pRT[:])
                pRep = psS.tile([K, K], FP32, tag="ps", name="pRep")
                nc.tensor.matmul(pRep[:], orow8[:], rdT[:], start=True, stop=True)
                hlhsT = small.tile([K, 18], FP32R, tag="hlhsT")
                nc.vector.tensor_tensor(
                    out=hlhsT[:, 0:8], in0=G1[:], in1=pRep[:], op=ALU.mult)
                nc.sync.dma_start(o_mats[b, :, 0:8], hlhsT[:, 0:8].bitcast(FP32))
                # g2 = rden * (p_bg . w2) ; s_ps = rden * rowsum(p_bg)
                pgw2 = small.tile([K, 1], FP32, tag="pgw2")
                nc.vector.tensor_tensor(out=scr8[:], in0=pbg[:], in1=w2r[:], op=ALU.mult)
                nc.vector.reduce_sum(pgw2[:], scr8[:], axis=AX.X)
                nc.vector.tensor_tensor(out=hlhsT[:, 8:9], in0=pgw2[:], in1=rdenK[:],
                                        op=ALU.mult)
                rsum8 = small.tile([K, 1], FP32, tag="rsum8")
                nc.vector.reduce_sum(rsum8[:], pbg[:], axis=AX.X)
                nc.vector.tensor_tensor(out=hlhsT[:, 9:10], in0=rsum8[:], in1=rdenK[:],
                                        op=ALU.mult)
                nc.vector.tensor_scalar_mul(hlhsT[:, 10:18], ident[0:8, 0:8], rdenK[:])

                # ================= hT group: [18, N] =================
                hsb = wide.tile([18, N], FP32, tag="hsb")
                for c4 in range(N4):
                    g = psG.tile([18, 512], FP32, tag="grp", name="grph")
                    nc.tensor.matmul(g[:], hlhsT[:], numT[0:8, 512 * c4 : 512 * (c4 + 1)],
                                     start=True, stop=True)
                    if c4 % 2 == 0:
                        nc.scalar.copy(hsb[:, 512 * c4 : 512 * (c4 + 1)], g[:])
                    else:
                        nc.vector.tensor_copy(hsb[:, 512 * c4 : 512 * (c4 + 1)], g[:])

                # ================= pack numT and hsb =================
                pNP = psS.tile([128, NC, 16], FP32, tag="ps", name="pNP")
                for c in range(NC):
                    nc.tensor.transpose(pNP[:, c, :],
                                        numT[:, 128 * c : 128 * (c + 1)].bitcast(FP32),
                                        ident[0:16, 0:16])
                npack = small.tile([128, NC, 16], FP32, tag="npack")
                nc.vector.tensor_copy(npack[:], pNP[:])
                pHP = psS.tile([128, NC, 18], FP32, tag="ps", name="pHP")
                for c in range(NC):
                    nc.tensor.transpose(pHP[:, c, :], hsb[:, 128 * c : 128 * (c + 1)],
                                        ident[0:18, 0:18])
                hpack = small.tile([128, NC, 18], FP32, tag="hpack")
                nc.scalar.copy(hpack[:], pHP[:])

                if phase <= 4:
                    continue
                numv = npack[:, :, 0:8]
                cdotv = npack[:, :, 8:16]
                hv = hpack[:, :, 0:8]
                qbgv = hpack[:, :, 8:9]
                mbgDv = hpack[:, :, 9:10]
                h2v = hpack[:, :, 10:18]

                def s16(tag):
                    return small.tile([128, NC], FP32, tag=tag, name=tag)

                scrP = small.tile([128, NC, K], FP32, tag="scrP")
                # nbg2 / dotrb / maxdot
                nbg2 = s16("nbg2")
                nc.vector.tensor_tensor(out=scrP[:], in0=numv, in1=hv, op=ALU.mult)
                nc.vector.reduce_sum(nbg2[:], scrP[:], axis=AX.X)
                dotrb = s16("dotrb")
                nc.vector.tensor_tensor(out=scrP[:], in0=numv, in1=h2v, op=ALU.mult)
                nc.vector.reduce_sum(dotrb[:], scrP[:], axis=AX.X)
                maxdot = s16("maxdot")
                nc.vector.reduce_max(maxdot[:], cdotv, axis=AX.X)

                # mu = (rowsum_raw - alpha*mbgD)/D
                mu = s16("mu")
                t1 = s16("t1")
                nc.vector.tensor_scalar_mul(t1[:], mbgDv.opt(), -alpha)
                nc.vector.tensor_tensor(out=t1[:], in0=t1[:], in1=qrpack[:, :, 1:2].opt(),
                                        op=ALU.add)
                nc.vector.tensor_scalar_mul(mu[:], t1[:], 1.0 / D)
                # sumsq_x = ssq - 2a*dotrb + a^2*nbg2
                sx = s16("sx")
                t2 = s16("t2")
                nc.vector.tensor_scalar_mul(t2[:], dotrb[:], -2.0 * alpha)
                nc.vector.tensor_tensor(out=t2[:], in0=t2[:], in1=ssq[:], op=ALU.add)
                nc.vector.tensor_scalar_mul(sx[:], nbg2[:], alpha * alpha)
                nc.vector.tensor_tensor(out=sx[:], in0=sx[:], in1=t2[:], op=ALU.add)
                # var = sx/D - mu^2
                var = s16("var")
                mu2 = s16("mu2")
                nc.vector.tensor_tensor(out=mu2[:], in0=mu[:], in1=mu[:], op=ALU.mult)
                nc.vector.tensor_scalar_mul(var[:], sx[:], 1.0 / D)
                nc.vector.tensor_tensor(out=var[:], in0=var[:], in1=mu2[:], op=ALU.subtract)

                def rsqrt16(x_ap, clamp_lo, tag, bias=0.0):
                    """1/max(sqrt(x+bias), clamp_lo) with one Newton refinement."""
                    if bias != 0.0:
                        xb = s16(tag + "_xb")
                        nc.vector.tensor_scalar_add(xb[:], x_ap, bias)
                        x_ap = xb[:]
                    s = s16(tag + "_s")
                    nc.scalar.activation(s[:], x_ap, AF.Sqrt)
                    sm = s16(tag + "_m")
                    nc.vector.tensor_scalar_max(sm[:], s[:], clamp_lo)
                    y = s16(tag + "_y")
                    nc.vector.reciprocal(y[:], sm[:])
                    # refine: y' = y*(1.5 - 0.5*x*y^2)  (x here = sm^2 ~ x+bias)
                    w = s16(tag + "_w")
                    nc.vector.tensor_tensor(out=w[:], in0=y[:], in1=y[:], op=ALU.mult)
                    nc.vector.tensor_tensor(out=w[:], in0=w[:], in1=sm[:], op=ALU.mult)
                    nc.vector.tensor_tensor(out=w[:], in0=w[:], in1=sm[:], op=ALU.mult)
                    nc.vector.tensor_scalar(out=w[:], in0=w[:], scalar1=-0.5,
                                            scalar2=1.5, op0=ALU.mult, op1=ALU.add)
                    nc.vector.tensor_tensor(out=y[:], in0=y[:], in1=w[:], op=ALU.mult)
                    return y

                rstd = rsqrt16(var[:], 0.0, "rstd", bias=LN_EPS)
                nb2c = s16("nb2c")
                nc.vector.tensor_scalar_max(nb2c[:], nbg2[:], 1e-30)
                rnbg = rsqrt16(nb2c[:], NORM_EPS, "rnbg")
                rnraw = rsqrt16(ssq[:], NORM_EPS, "rnraw")

                # stats: cmpvals / diagvals
                statsT = small.tile([128, 32], FP32, tag="statsT")
                nc.vector.tensor_tensor(out=statsT[:, 0:NC], in0=maxdot[:], in1=rnraw[:],
                                        op=ALU.mult)
                dg = s16("dg")
                nc.vector.tensor_tensor(out=dg[:], in0=nbg2[:], in1=rnbg[:], op=ALU.mult)
                nc.vector.tensor_tensor(out=dg[:], in0=dg[:], in1=rnbg[:], op=ALU.mult)
                nc.vector.tensor_tensor(out=statsT[:, 16:32], in0=dg[:], in1=dg[:],
                                        op=ALU.mult)
                nc.sync.dma_start(o_stats[b], statsT[:])

                # c2 = rnbg * numer (packed) -> M = sum c2^T c2
                c2r = small.tile([128, NC, K], FP32R, tag="c2r")
                for c in range(NC):
                    nc.vector.tensor_scalar_mul(c2r[:, c, :], numv[:, c, :].opt(),
                                                rnbg[:, c : c + 1])
                pM = psS.tile([K, K], FP32, tag="ps", name="pM")
                for c in range(NC):
                    nc.tensor.matmul(pM[:], c2r[:, c, :], c2r[:, c, :],
                                     start=(c == 0), stop=(c == NC - 1))
                Msb = small.tile([K, K], FP32, tag="Msb")
                nc.vector.tensor_copy(Msb[:], pM[:])
                nc.sync.dma_start(o_mats[b, :, 8:16], Msb[:])

                if phase <= 5:
                    continue
                # ---- sim / s_sem ----
                if not general_ln:
                    qdx = s16("qdx")
                    nc.vector.tensor_scalar_mul(qdx[:], qbgv.opt(), -alpha)
                    nc.vector.tensor_tensor(out=qdx[:], in0=qdx[:],
                                            in1=qrpack[:, :, 0:1].opt(), op=ALU.add)
                    # sim_raw = (qdx - mu*S_w2)*rstd + C
                    simr = s16("simr")
                    nc.vector.tensor_scalar_mul(simr[:], mu[:], lns[:, 0:1])
                    nc.vector.tensor_tensor(out=simr[:], in0=qdx[:], in1=simr[:],
                                            op=ALU.subtract)
                    nc.vector.tensor_tensor(out=simr[:], in0=simr[:], in1=rstd[:],
                                            op=ALU.mult)
                    # ffg2 = D*var*rstd^2
                    ffg2 = s16("ffg2")
                    nc.vector.tensor_tensor(out=ffg2[:], in0=rstd[:], in1=rstd[:],
                                            op=ALU.mult)
                    nc.vector.tensor_tensor(out=ffg2[:], in0=ffg2[:], in1=var[:],
                                            op=ALU.mult)
                    nc.vector.tensor_scalar_mul(ffg2[:], ffg2[:], float(D))

                # ================= f_bg / x / f_fg =================
                ffg2_acc = s16("ffg2acc") if general_ln else None
                simr_g = s16("simr_g") if general_ln else None
                for c in range(NC):
                    pfb = psF.tile([128, D], FP32, tag="pfb")
                    nc.tensor.matmul(pfb[:], numT[0:8, 128 * c : 128 * (c + 1)],
                                     pbg_s[:], start=True, stop=True)
                    fbg_sb = stage.tile([128, D], FP32, tag="fbg_sb")
                    nc.scalar.copy(fbg_sb[:], pfb[:])
                    nc.sync.dma_start(o_fbg[b, 128 * c : 128 * (c + 1), :], fbg_sb[:])
                    x_sb = stage.tile([128, D], FP32, tag="x_sb")
                    if alpha == 1.0:
                        nc.vector.tensor_tensor(out=x_sb[:], in0=fr[:, c, :],
                                                in1=fbg_sb[:], op=ALU.subtract)
                    else:
                        fbs = stage.tile([128, D], FP32, tag="fbs")
                        nc.scalar.mul(fbs[:], fbg_sb[:], float(alpha))
                        nc.vector.tensor_tensor(out=x_sb[:], in0=fr[:, c, :],
                                                in1=fbs[:], op=ALU.subtract)
                    ffg_sb = stage.tile([128, D], FP32, tag="ffg_sb")
                    nc.vector.tensor_scalar(out=ffg_sb[:], in0=x_sb[:],
                                            scalar1=mu[:, c : c + 1],
                                            scalar2=rstd[:, c : c + 1],
                                            op0=ALU.subtract, op1=ALU.mult)
                    if general_ln:
                        nc.vector.tensor_tensor(out=ffg_sb[:], in0=ffg_sb[:], in1=lnw[:],
                                                op=ALU.mult)
                        nc.vector.tensor_tensor(out=ffg_sb[:], in0=ffg_sb[:], in1=lnb[:],
                                                op=ALU.add)
                        sq2 = scr.tile([128, D], FP32, tag="sq2")
                        nc.scalar.activation(sq2[:], ffg_sb[:], AF.Square,
                                             accum_out=ffg2_acc[:, c : c + 1])
                        sq3 = scr.tile([128, D], FP32, tag="sq3")
                        nc.vector.tensor_tensor(out=sq3[:], in0=ffg_sb[:], in1=fqn[:],
                                                op=ALU.mult)
                        nc.vector.reduce_sum(simr_g[:, c : c + 1], sq3[:], axis=AX.X)
                    nc.sync.dma_start(o_ffg[b, 128 * c : 128 * (c + 1), :], ffg_sb[:])

                if general_ln:
                    simr = simr_g
                    ffg2 = ffg2_acc

                rnfg = rsqrt16(ffg2[:], NORM_EPS, "rnfg")
                sim = s16("sim")
                nc.vector.tensor_tensor(out=sim[:], in0=simr[:], in1=rnfg[:],
                                        op=ALU.mult)
                ssem = small.tile([128, NC], FP32, tag="ssem")
                nc.scalar.activation(ssem[:], sim[:], AF.Sigmoid, scale=1.0 / TAU)
                nc.sync.dma_start(o_ssem[b], ssem[:])

    nc.finalize()
    return nc


def _get_prog(alpha: float, general_ln: bool):
    import os
    phase = int(os.environ.get("BASS_KERNEL_PHASE", "99"))
    key = (float(alpha), bool(general_ln), phase)
    if key not in _PROG_CACHE:
        _PROG_CACHE[key] = _build(float(alpha), bool(general_ln), phase)
    return _PROG_CACHE[key]


def kernel(f_raw, f_q, gt_mask, q_bg, alpha, ln_w, ln_b):
    f_raw = np.asarray(f_raw, dtype=np.float32)
    f_q = np.asarray(f_q, dtype=np.float32)
    gt_mask = np.asarray(gt_mask)
    q_bg = np.asarray(q_bg, dtype=np.float32)
    alpha_f = float(np.asarray(alpha, dtype=np.float32))
    ln_w = np.asarray(ln_w, dtype=np.float32)
    ln_b = np.asarray(ln_b, dtype=np.float32)

    general_ln = not (np.all(ln_w == 1.0) and np.all(ln_b == 0.0))

    # host precompute
    fqn = f_q / np.maximum(np.linalg.norm(f_q, axis=-1, keepdims=True), NORM_EPS)
    w2 = ln_w[None, :] * fqn                      # [B, D]
    Sw2 = w2.sum(axis=1)                          # [B]
    Cb = (ln_b[None, :] * fqn).sum(axis=1)        # [B]

    qmat_all = np.zeros((B, D, 10), dtype=np.float32)
    qmat_all[:, :, 0:8] = (q_bg.T / np.sqrt(np.float32(D)))[None]
    qmat_all[:, :, 8] = w2
    qmat_all[:, :, 9] = 1.0

    ident = np.eye(128, dtype=np.float32)
    ones128 = np.ones((128, 1), dtype=np.float32)
    orow8 = np.ones((1, 8), dtype=np.float32)

    nc = _get_prog(alpha_f, general_ln)
    in_maps = []
    for core in range(NCORES):
        bs = slice(PB * core, PB * (core + 1))
        lns = np.zeros((PB, 2, 128), dtype=np.float32)
        lns[:, 0, :] = Sw2[bs][:, None]
        lns[:, 1, :] = Cb[bs][:, None]
        m = {
            "f_raw": np.ascontiguousarray(f_raw[bs]),
            "qmat": np.ascontiguousarray(qmat_all[bs]),
            "w2rep": np.ascontiguousarray(
                np.broadcast_to(w2[bs][:, None, :], (PB, K, D))),
            "lnsc": lns,
            "ident": ident,
            "ones128": ones128,
            "onesrow8": orow8,
        }
        if general_ln:
            m["lnwrep"] = np.ascontiguousarray(
                np.broadcast_to(ln_w[None, :], (128, D)))
            m["lnbrep"] = np.ascontiguousarray(
                np.broadcast_to(ln_b[None, :], (128, D)))
            m["fqnrep"] = np.ascontiguousarray(
                np.broadcast_to(fqn[bs][:, None, :], (PB, 128, D)))
        in_maps.append(m)

    global _LAST_IN_MAPS
    _LAST_IN_MAPS = in_maps
    res = None
    last_err = None
    for _attempt in range(3):
        try:
            res = run_bass_kernel_spmd(nc, in_maps, list(range(NCORES))).results
            break
        except Exception as e:  # transient NRT device wedges recover on retry
            last_err = e
    if res is None:
        raise last_err

    f_fg = np.concatenate([res[i]["f_fg"] for i in range(NCORES)], axis=0)
    f_bg = np.concatenate([res[i]["f_bg"] for i in range(NCORES)], axis=0)
    p_bg = np.concatenate([res[i]["p_bg"] for i in range(NCORES)], axis=0)
    ssem_p = np.concatenate([res[i]["ssem"] for i in range(NCORES)], axis=0)
    stats = np.concatenate([res[i]["stats"] for i in range(NCORES)], axis=0)
    mats = np.concatenate([res[i]["mats"] for i in range(NCORES)], axis=0)

    # unpack s_sem: packed [B, 128, NC] with n = 128*c + p
    s_sem = ssem_p.transpose(0, 2, 1).reshape(B, N).astype(np.float32)

    # losses (host reductions of device partials)
    cmp_sum = np.float64(stats[:, :, 0:NC].sum(dtype=np.float64))
    loss_compact = np.float32(1.0 - cmp_sum / (B * N))

    orth = np.float64(0.0)
    for b in range(B):
        Gs = np.float64(mats[b, :, 0:8])
        M = np.float64(mats[b, :, 8:16])
        GM = Gs @ M
        orth += np.trace(GM @ GM) - stats[b, :, 16:32].sum(dtype=np.float64)
    loss_orth = np.float32(orth / (B * N * N))

    mk = gt_mask.astype(np.float64)
    cnt = mk.sum()
    neg_log = -np.log(np.maximum(s_sem.astype(np.float64), EPS))
    if cnt > 0:
        loss_align = np.float32((neg_log * mk).sum() / max(cnt, 1.0))
    else:
        loss_align = np.float32(0.0)

    return (f_fg, f_bg, p_bg, s_sem,
            np.float32(loss_compact), np.float32(loss_orth), np.float32(loss_align))
